# revision 33
# baseline (speedup 1.0000x reference)
"""Linear-attention (ELU+1 feature map) Bass kernel for TRN2, 8 NeuronCores.

Problem: B=8, N=4096, C=512, 8 heads, d=64.
  q = x @ Wq.T;  kv = x @ Wkv.T -> k, v
  Q = elu(q)+1; K = elu(k)+1
  KV[h,d,v] = sum_s K[s,h,d] v[s,h,v]
  Z[l,h]  = 1/(Q[l,h,:] . sum_s K[s,h,:] + eps)
  out[l,h,v] = sum_d Q[l,h,d] KV[h,d,v] * Z[l,h]
  (the reference's /N on v and *N on out cancel; eps is negligible vs den~1e5)

Sharding: data-parallel over B — core b computes batch b. No collectives.

Single-core dataflow (x_b [4096, 512]), all matmuls bf16 (fp32 PSUM accum):
  phase 0: DMA Wq/Wkv, TensorE-transpose to WqT/WkvT chunks (bf16).
  phase 1 (per 512-token macro-tile):
    - DMA 4x [128,512] f32 x sub-tiles; PE-transpose -> psum -> bf16 xT
    - qT[o,tok] = WqT.T @ xT   (N=512)  -> elu+1 -> QT (bf16, resident)
    - k,v[tok,o] = xT.T @ WkvT (N=512)  -> elu+1 -> K (bf16); v -> V_aug
    - V_aug has a ones-column per 2-head chunk; KV_aug += K_chunk.T @ V_aug_chunk
      (N=129) accumulates KV and Ksum into persistent PSUM over all tiles.
  elu+1 epilogue (exp(min(x,0)) == min(exp(x),1)):
    e = exp(x) [ACT], r = relu(x) [ACT for q / DVE for k],
    out = (e min 1) + r  [fused scalar_tensor_tensor, DVE bf16 2x mode].
  phase 2 (per 128-token tile):
    - [num | den] = QT_chunk.T @ [KV_clean_chunk | Ksum cols]  (N=130)
    - out = num * recip(den)  -> DMA out.
"""
import contextlib
import os
import sys

for _p in ("/opt/trn_rl_repo", "/root/.axon_site/_ro/trn_rl_repo"):
    if os.path.isdir(_p) and _p not in sys.path:
        sys.path.insert(0, _p)

import numpy as np

import concourse.bass as bass
import concourse.tile as tile
from concourse import bacc, masks, mybir
from concourse.bass_utils import run_bass_kernel_spmd

dt = mybir.dt
AF = mybir.ActivationFunctionType
ALU = mybir.AluOpType

N_CORES = 8
B, N, C = 8, 4096, 512
H, D = 8, 64
P = 128          # partitions / tile row count
CH = C // P      # 4 contraction chunks
NT = N // P      # 32 token tiles
TM = 4           # token tiles per macro-tile
NM = NT // TM    # 8 macro-tiles
W_AUG = P + 1    # 129: per-chunk KV columns incl. ones column
W2 = P + 2       # 130: phase-2 rhs cols (128 num + 2 den)


def _build_phase0(nc, tc, sb_w, ps, ident, wq_ext, wkv_ext):
    """DMA weights, transpose to [c, o] chunk layout in bf16."""
    wq_sb = sb_w.tile([P, CH, C], dt.float32, name="wq_sb")
    nc.sync.dma_start(wq_sb[:], wq_ext[:].rearrange("(a p) c -> p a c", p=P))
    wkv_sb = sb_w.tile([P, 2 * CH, C], dt.float32, name="wkv_sb")
    nc.sync.dma_start(wkv_sb[:], wkv_ext[:].rearrange("(a p) c -> p a c", p=P))

    wqT = [sb_w.tile([P, C], dt.bfloat16, name=f"wqT{ci}") for ci in range(CH)]
    wkvT = [sb_w.tile([P, 2 * C], dt.bfloat16, name=f"wkvT{ci}")
            for ci in range(CH)]
    for ci in range(CH):
        pt = ps.tile([P, C], dt.float32, name="pt_w", tag="ptx", bufs=2)
        for oj in range(CH):
            nc.tensor.transpose(
                pt[:, oj * P:(oj + 1) * P], wq_sb[:, oj, ci * P:(ci + 1) * P],
                ident[:],
            )
        nc.vector.tensor_copy(wqT[ci][:], pt[:])
        for half in range(2):
            pt2 = ps.tile([P, C], dt.float32, name="pt_w2", tag="ptx", bufs=2)
            for oj in range(CH):
                nc.tensor.transpose(
                    pt2[:, oj * P:(oj + 1) * P],
                    wkv_sb[:, half * CH + oj, ci * P:(ci + 1) * P],
                    ident[:],
                )
            nc.vector.tensor_copy(wkvT[ci][:, half * C:(half + 1) * C], pt2[:])
    return wqT, wkvT


def _elu1(nc, pool, out_ap, src_psum, variant, ablate=()):
    """out = elu(src)+1 = (exp(src) min 1) + relu(src), all from PSUM once.

    exp(min(x,0)) == min(exp(x),1). Engine split by variant:
      "q": exp ACT, relu ACT, fused clamp+add DVE (bf16 2x mode)
      "k": exp ACT, relu DVE (balances ACT/DVE load), fused on DVE
    """
    if "elu" in ablate:
        nc.vector.tensor_copy(out_ap, src_psum)
        return
    p, f = src_psum.shape[0], src_psum.shape[1]
    e = pool.tile([p, f], dt.bfloat16, name="elu_e", tag="elu_e", bufs=4)
    nc.scalar.activation(e[:], src_psum, AF.Exp)
    r = pool.tile([p, f], dt.bfloat16, name="elu_r", tag="elu_r", bufs=4)
    if variant == "q":
        nc.scalar.activation(r[:], src_psum, AF.Relu)
    else:
        nc.vector.tensor_scalar_max(r[:], src_psum, 0.0)
    nc.vector.scalar_tensor_tensor(
        out_ap, e[:], 1.0, r[:], op0=ALU.min, op1=ALU.add
    )


def build_nc(loop_reps=1, ablate=()):
    nc = bacc.Bacc("TRN2", target_bir_lowering=False, debug=False,
                   num_devices=N_CORES)
    x_ext = nc.dram_tensor("x", (N, C), dt.float32, kind="ExternalInput")
    wq_ext = nc.dram_tensor("Wq", (C, C), dt.float32, kind="ExternalInput")
    wkv_ext = nc.dram_tensor("Wkv", (2 * C, C), dt.float32, kind="ExternalInput")
    out_ext = nc.dram_tensor("out", (N, C), dt.float32, kind="ExternalOutput")

    with tile.TileContext(nc) as tc:
        with tc.tile_pool(name="sb_w", bufs=1) as sb_w, \
             tc.tile_pool(name="sb_qt", bufs=1) as sb_qt, \
             tc.tile_pool(name="sb", bufs=1) as sb, \
             tc.tile_pool(name="ps", bufs=1, space="PSUM") as ps, \
             tc.tile_pool(name="ps_acc", bufs=1, space="PSUM") as ps_acc:

            ident = sb_w.tile([P, P], dt.float32, name="ident")
            masks.make_identity(nc, ident[:])

            rep_ctx = (tc.For_i(0, loop_reps, 1) if loop_reps > 1
                       else contextlib.nullcontext())
            with rep_ctx:
                _build_body(nc, tc, sb_w, sb_qt, sb, ps, ps_acc, ident,
                            x_ext, wq_ext, wkv_ext, out_ext, ablate)

    nc.compile()
    return nc


def _build_body(nc, tc, sb_w, sb_qt, sb, ps, ps_acc, ident,
                x_ext, wq_ext, wkv_ext, out_ext, ablate=()):
    wqT, wkvT = _build_phase0(nc, tc, sb_w, ps, ident, wq_ext, wkv_ext)

    # resident Q^T, bf16: 4 chunks [128, 4096]
    qT = [sb_qt.tile([P, N], dt.bfloat16, name=f"qT{ci}")
          for ci in range(CH)]
    # persistent KV accumulation PSUM: 2 banks, 2 chunks per bank
    kv_ps = ps_acc.tile([P, 2, 512], dt.float32, name="kv_ps")
    # Clear each bank once with a K=1 zero matmul: start=True clears
    # has_written for the WHOLE bank, so per-chunk groups sharing a
    # bank must not each open with start=True. After this, every
    # accumulating matmul uses start=False (accumulate-where-set).
    zlhs = sb_w.tile([1, P], dt.bfloat16, name="zlhs")
    zrhs = sb_w.tile([1, 512], dt.bfloat16, name="zrhs")
    nc.vector.memset(zlhs[:], 0.0)
    nc.vector.memset(zrhs[:], 0.0)
    for bk in range(2):
        nc.tensor.matmul(kv_ps[:, bk, :], zlhs[:], zrhs[:],
                         start=True, stop=True)

    # ---------------- phase 1 ----------------
    for mi in range(NM):
        xs = []
        for tj in range(TM):
            xt_in = sb.tile([P, C], dt.float32, name="x_in",
                            tag="x_in", bufs=32)
            t0 = (mi * TM + tj) * P
            nc.sync.dma_start(xt_in[:], x_ext[t0:t0 + P, :])
            xs.append(xt_in)

        # PE-transpose f32 x chunks -> psum -> bf16 xT in SBUF
        xTc = []
        for ci in () if "tpose" in ablate else range(CH):
            pt = ps.tile([P, TM * P], dt.float32, name="pt_x",
                         tag="ptx", bufs=2)
            for tj in range(TM):
                nc.tensor.transpose(
                    pt[:, tj * P:(tj + 1) * P],
                    xs[tj][:, ci * P:(ci + 1) * P], ident[:],
                )
            xc = sb.tile([P, TM * P], dt.bfloat16, name="xT",
                         tag="xT", bufs=8)
            nc.vector.tensor_copy(xc[:, 0:TM * P // 2], pt[:, 0:TM * P // 2])
            nc.scalar.copy(xc[:, TM * P // 2:], pt[:, TM * P // 2:])
            xTc.append(xc)

        # q^T chunks: [o 128, 512 tok]; fused elu on DVE
        for oj in () if "proj" in ablate else range(CH):
            pq = ps.tile([P, TM * P], dt.float32, name="pq",
                         tag="pq", bufs=2)
            for ci in range(CH):
                nc.tensor.matmul(
                    pq[:], wqT[ci][:, oj * P:(oj + 1) * P], xTc[ci][:],
                    start=(ci == 0), stop=(ci == CH - 1),
                )
            _elu1(nc, sb, qT[oj][:, mi * TM * P:(mi + 1) * TM * P],
                  pq[:], "q", ablate)

        # k, v (token-major) + KV accumulation
        for tj in () if "proj" in ablate else range(TM):
            pk = ps.tile([P, C], dt.float32, name="pk", tag="pkv", bufs=2)
            pv = ps.tile([P, C], dt.float32, name="pv", tag="pkv", bufs=2)
            for ci in range(CH):
                nc.tensor.matmul(
                    pk[:], xTc[ci][:, tj * P:(tj + 1) * P],
                    wkvT[ci][:, 0:C],
                    start=(ci == 0), stop=(ci == CH - 1),
                )
            for ci in range(CH):
                nc.tensor.matmul(
                    pv[:], xTc[ci][:, tj * P:(tj + 1) * P],
                    wkvT[ci][:, C:2 * C],
                    start=(ci == 0), stop=(ci == CH - 1),
                )
            ksb = sb.tile([P, C], dt.bfloat16, name="ksb",
                          tag="ksb", bufs=3)
            _elu1(nc, sb, ksb[:], pk[:], "k", ablate)
            vaug = sb.tile([P, CH * W_AUG], dt.bfloat16, name="vaug",
                           tag="vaug", bufs=3)
            vv = vaug[:].rearrange("p (c w) -> p c w", w=W_AUG)
            nc.vector.tensor_copy(
                vv[:, :, 0:P], pv[:].rearrange("p (c w) -> p c w", w=P)
            )
            nc.vector.memset(vv[:, :, P:W_AUG], 1.0)

            last = (mi == NM - 1 and tj == TM - 1)
            if "kv" not in ablate:
                for c in range(CH):
                    nc.tensor.matmul(
                        kv_ps[:, c // 2,
                              (c % 2) * W_AUG:(c % 2 + 1) * W_AUG],
                        ksb[:, c * P:(c + 1) * P],
                        vaug[:, c * W_AUG:(c + 1) * W_AUG],
                        start=False, stop=last,
                        skip_group_check=True,
                    )

    # ---------------- phase boundary ----------------
    # kvdw bf16 [128, 4*130]: per chunk [KV diag blocks (128) | ksum 2 cols]
    kvdw = sb_w.tile([P, CH * W2], dt.bfloat16, name="kvdw")
    nc.vector.memset(kvdw[:], 0.0)
    for c in range(CH):
        bk, co = c // 2, (c % 2) * W_AUG
        o2 = c * W2
        nc.vector.tensor_copy(
            kvdw[0:D, o2:o2 + D], kv_ps[0:D, bk, co:co + D])
        nc.vector.tensor_copy(
            kvdw[D:P, o2 + D:o2 + P], kv_ps[D:P, bk, co + D:co + P])
        nc.vector.tensor_copy(
            kvdw[0:D, o2 + P:o2 + P + 1],
            kv_ps[0:D, bk, co + P:co + W_AUG])
        nc.vector.tensor_copy(
            kvdw[D:P, o2 + P + 1:o2 + W2],
            kv_ps[D:P, bk, co + P:co + W_AUG])

    # ---------------- phase 2 ----------------
    if "ph2" in ablate:
        dummy = sb.tile([P, C], dt.float32, name="dummy_o", tag="osb", bufs=3)
        nc.vector.memset(dummy[:], 0.0)
        nc.sync.dma_start(out_ext[0:P, :], dummy[:])
        return
    for t in range(NT):
        if t % 2 == 0:
            om = sb.tile([P, 2, C], dt.float32, name="om", tag="osb", bufs=4)
        pnA = ps.tile([P, 2, W2], dt.float32, name="pnA", tag="pq", bufs=2)
        pnB = ps.tile([P, 2, W2], dt.float32, name="pnB", tag="pkv", bufs=2)
        for c in range(CH):
            pb = pnA if c < 2 else pnB
            nc.tensor.matmul(
                pb[:, c % 2, :],
                qT[c][:, t * P:(t + 1) * P],
                kvdw[:, c * W2:(c + 1) * W2],
                start=True, stop=True,
            )
        osb = om[:, t % 2]
        if "ph2dve" in ablate:
            nc.vector.tensor_copy(osb[:, 0:256], pnA[:, :, 0:P])
            nc.vector.tensor_copy(osb[:, 256:512], pnB[:, :, 0:P])
        else:
            zr = sb.tile([P, H], dt.float32, name="zr", tag="zr", bufs=3)
            for b, pb in enumerate((pnA, pnB)):
                nc.vector.reciprocal(
                    zr[:, b * 4:(b + 1) * 4], pb[:, :, P:W2])
                nc.vector.tensor_tensor(
                    osb[:, b * 256:(b + 1) * 256].rearrange(
                        "p (c h w) -> p c h w", c=2, w=D),
                    pb[:, :, 0:P].rearrange("p c (h w) -> p c h w", w=D),
                    zr[:, b * 4:(b + 1) * 4].rearrange("p (c h) -> p c h", c=2)
                    .broadcast_to((P, 2, 2, D)),
                    op=ALU.mult,
                )
        if "ph2dma" not in ablate and t % 2 == 1:
            r0 = (t - 1) * P
            nc.sync.dma_start(
                out_ext[r0:r0 + 2 * P, :].rearrange("(a p) c -> p a c", p=P),
                om[:])
    if "ph2dma" in ablate:
        nc.sync.dma_start(out_ext[0:P, :], osb[:])


_NC_CACHE = None


def _get_nc():
    global _NC_CACHE
    if _NC_CACHE is None:
        _NC_CACHE = build_nc()
    return _NC_CACHE


def run(inputs, trace=False, **kw):
    x = np.ascontiguousarray(inputs["x"], dtype=np.float32)
    wq = np.ascontiguousarray(inputs["Wq"], dtype=np.float32)
    wkv = np.ascontiguousarray(inputs["Wkv"], dtype=np.float32)
    nc = _get_nc()
    in_maps = [{"x": x[b], "Wq": wq, "Wkv": wkv} for b in range(N_CORES)]
    res = run_bass_kernel_spmd(nc, in_maps, core_ids=list(range(N_CORES)),
                               trace=trace, **kw)
    out = np.stack([res.results[b]["out"] for b in range(N_CORES)], axis=0)
    return out, res


def kernel(**inputs):
    out, _ = run(inputs)
    return out
